# revision 1
# baseline (speedup 1.0000x reference)
"""Trainium2 Bass kernel for nn_CrossModalAttention (B=4, S=2048, H=2048, single head).

Sharding: 8 cores = 4 batches x 2 query-halves (K/V projections duplicated
across the pair; the pairwise-exchange variant was tried and the D2D link cost
~200us per 8MB AllReduce, stalling concurrent DMAs - a wash).

Single-core dataflow, all bf16 staging (same PE rate as f32r at moving>=256,
half the DMA/SBUF of f32r):
  A  v' = value @ WvT            -> v_st (HBM, bf16)   [k-rope on DVE]
  B  kT' = WkT.T @ krot + bk     -> kTp_sb (SBUF!)     [q-rope on DVE]
  C  qT' = WqT.T @ qrot + bq     -> qTp_sb (SBUF)
  D  expT = exp(scale * kTp.T-slices @ qTp); den via PE ones-matmul  (DMA-free)
  E  ctxT = (v-strips @ expT) * (1/den) + bv
  F  outT = WoT.T @ ctxT + bo -> HBM

No HBM staging round-trips except v (SBUF cannot hold v+kTp together in B).
Overlapping pool lifetimes (krot A-B, qrot B-C, kTp B-D) use manual
__enter__/__exit__ since nested `with` cannot express chains.
"""

import sys

for _p in ("/opt/trn_rl_repo",):
    if _p not in sys.path:
        sys.path.append(_p)

import numpy as np

B, S, H = 4, 2048, 2048
P = 128
HO = H // P            # 16 h-tiles
SQ = S // 2            # 1024 query rows per core
SK = S                 # 2048 key rows (full, duplicated per pair)
NC_ = 8
SCALE = 1.0 / float(np.sqrt(H))

_PROG = None
_TRACE = False
LAST_RES = None


def _emit(nc, tile, mybir):
    F32 = mybir.dt.float32
    F16 = mybir.dt.float16
    F32R = mybir.dt.float32r
    BF16 = mybir.dt.bfloat16
    Exp = mybir.ActivationFunctionType.Exp
    Ident = mybir.ActivationFunctionType.Identity

    dram = {}
    def din(name, shape, dt=F32):
        dram[name] = nc.dram_tensor(name, list(shape), dt, kind="ExternalInput").ap()
    din("qT", (H, SQ), BF16)
    din("kT", (H, SK), BF16)
    din("vT", (H, SK), BF16)
    din("cos_q", (H // 2, SQ), F16); din("sin_q", (H // 2, SQ), F16)
    din("cos_k", (H // 2, SK), F16); din("sin_k", (H // 2, SK), F16)
    din("wq", (H, H), BF16); din("wk", (H, H), BF16)
    din("wv", (H, H), BF16); din("wo", (H, H), BF16)
    din("bq", (H,)); din("bk", (H,)); din("bv", (H,)); din("bo", (H,))
    din("ones_col", (P, 1), BF16); din("ones_row", (1, P), F32R)
    outT = nc.dram_tensor("outT", [H, SQ], F32, kind="ExternalOutput").ap()

    def strip_ap(src2d, col0, width):
        return src2d[:, col0:col0 + width].rearrange("(o p) s -> p o s", p=P)

    def ld_strip(pool, src2d, col0, width, tag):
        rows = src2d.shape[0]
        t = pool.tile([P, rows // P, width], src2d.dtype, name=tag)
        nc.sync.dma_start(out=t[:], in_=strip_ap(src2d, col0, width))
        return t

    with tile.TileContext(nc) as tc:
        from contextlib import ExitStack
        with ExitStack() as ctx:
            glob = ctx.enter_context(tc.tile_pool(name="glob", bufs=1))
            dstage = ctx.enter_context(tc.tile_pool(name="dram", bufs=1, space="DRAM"))
            psum = ctx.enter_context(tc.tile_pool(name="psum", bufs=6, space="PSUM"))

            v_st = dstage.tile([SK, H], BF16, name="v_st")

            bias_sb = {}
            for bn in ("bq", "bk", "bv", "bo"):
                bias_sb[bn] = glob.tile([P, HO], F32, name=f"{bn}_sb")
            ones_col = glob.tile([P, 1], BF16, name="ones_col")
            ones_row = glob.tile([1, P], F32R, name="ones_row")
            recip_bc = glob.tile([P, SQ], F32, name="recip_bc")

            def load_globals():
                for bn in ("bq", "bk", "bv", "bo"):
                    nc.sync.dma_start(out=bias_sb[bn][:],
                                      in_=dram[bn].rearrange("(t p) -> p t", p=P))
                nc.sync.dma_start(out=ones_col[:], in_=dram["ones_col"])
                nc.sync.dma_start(out=ones_row[:], in_=dram["ones_row"])

            def rope_chunk(dst, xsrc, cos_ap, sin_ap, pool, tag, c0, cw):
                HH = HO // 2
                lo, hi = slice(0, HH), slice(HH, HO)
                if True:
                    x = pool.tile([P, HO, cw], BF16, name=f"{tag}_in")
                    nc.sync.dma_start(out=x[:], in_=strip_ap(xsrc, c0, cw))
                    cs = pool.tile([P, HH, cw], F16, name=f"{tag}_cos")
                    nc.sync.dma_start(out=cs[:], in_=strip_ap(cos_ap, c0, cw))
                    sn = pool.tile([P, HH, cw], F16, name=f"{tag}_sin")
                    nc.sync.dma_start(out=sn[:], in_=strip_ap(sin_ap, c0, cw))
                    tmp = pool.tile([P, HH, cw], BF16, name=f"{tag}_tmp")
                    d = dst[:, :, c0:c0 + cw]
                    nc.vector.tensor_mul(d[:, lo, :], x[:, lo, :], cs[:])
                    nc.vector.tensor_mul(tmp[:], x[:, hi, :], sn[:])
                    nc.vector.tensor_sub(d[:, lo, :], d[:, lo, :], tmp[:])
                    nc.vector.tensor_mul(d[:, hi, :], x[:, hi, :], cs[:])
                    nc.vector.tensor_mul(tmp[:], x[:, lo, :], sn[:])
                    nc.vector.tensor_add(d[:, hi, :], d[:, hi, :], tmp[:])

            def rope_into(dst, xsrc, cos_ap, sin_ap, pool, tag, cw=128):
                for c0 in range(0, xsrc.shape[1], cw):
                    rope_chunk(dst, xsrc, cos_ap, sin_ap, pool, tag, c0, cw)

            # overlapping-lifetime pools (manual enter/exit):
            krot_cm = tc.tile_pool(name="krot", bufs=1)   # A..B
            krotp = krot_cm.__enter__()
            krot_sb = krotp.tile([P, HO, SK], BF16, name="krot_sb")     # 64KB

            # ---- Phase A: v-proj (full keys) + k-rope on DVE ----
            with tc.tile_pool(name="vT", bufs=1) as vTp, \
                 tc.tile_pool(name="wv", bufs=2) as wvp, \
                 tc.tile_pool(name="p1e", bufs=4) as p1e, \
                 tc.tile_pool(name="krope", bufs=2) as krope:
                vT_sb = vTp.tile([P, HO, SK], BF16, name="vT_sb")       # 64KB
                nc.sync.dma_start(out=vT_sb[:, :, 0:512],
                                  in_=strip_ap(dram["vT"], 0, 512))
                for oc in range(4):
                    wvq = wvp.tile([P, HO, 512], BF16, name="wv_q")
                    nc.sync.dma_start(out=wvq[:], in_=strip_ap(dram["wv"], oc * 512, 512))
                    for st in range(SK // P):           # 16 strips
                        ps = psum.tile([P, 512], F32, name="ps_mm")
                        for h in range(HO):
                            nc.tensor.matmul(
                                ps[:], vT_sb[:, h, st * P:(st + 1) * P],
                                wvq[:, h, :],
                                start=(h == 0), stop=(h == HO - 1))
                        ev = p1e.tile([P, 512], BF16, name="p1_ev")
                        nc.scalar.copy(ev[:], ps[:])
                        nc.sync.dma_start(
                            out=v_st[st * P:(st + 1) * P, oc * 512:(oc + 1) * 512],
                            in_=ev[:])
                        if oc == 0 and st == 0:
                            for hf in range(1, 4):
                                nc.sync.dma_start(
                                    out=vT_sb[:, :, hf * 512:(hf + 1) * 512],
                                    in_=strip_ap(dram["vT"], hf * 512, 512))
                # k-rope on DVE (overlaps the PE loop above)
                rope_into(krot_sb, dram["kT"], dram["cos_k"], dram["sin_k"],
                          krope, "kr")
                load_globals()

            # kTp: written in B (scalar evictions), read in D
            kTp_cm = tc.tile_pool(name="kTp", bufs=1, side="right")
            kTpp = kTp_cm.__enter__()
            kTp_sb = kTpp.tile([P, HO, SK], BF16, name="kTp_sb")        # 64KB
            # qrot: written in B (DVE), read in C
            qrot_cm = tc.tile_pool(name="qrot", bufs=1, side="right")
            qrotp = qrot_cm.__enter__()
            qrot_sb = qrotp.tile([P, HO, SQ], BF16, name="qrot_sb")     # 32KB

            # ---- Phase B: k-proj (+bk) -> kTp_sb + q-rope on DVE ----
            with tc.tile_pool(name="wk", bufs=2) as wkp, \
                 tc.tile_pool(name="qrope", bufs=2) as qrope:
                for eg in range(8):
                    wke = wkp.tile([P, HO, 256], BF16, name="wk_e")
                    nc.sync.dma_start(out=wke[:], in_=strip_ap(dram["wk"], eg * 256, 256))
                    for obl in range(2):
                        ob = eg * 2 + obl
                        for kc in range(4):
                            ps = psum.tile([P, 512], F32, name="ps_mm")
                            for h in range(HO):
                                nc.tensor.matmul(
                                    ps[:], wke[:, h, obl * P:(obl + 1) * P],
                                    krot_sb[:, h, kc * 512:(kc + 1) * 512],
                                    start=(h == 0), stop=(h == HO - 1))
                            nc.scalar.activation(
                                kTp_sb[:, ob, kc * 512:(kc + 1) * 512],
                                ps[:], Ident, bias=bias_sb["bk"][:, ob:ob + 1])
                    rope_chunk(qrot_sb, dram["qT"], dram["cos_q"],
                               dram["sin_q"], qrope, "qr", eg * 128, 128)

            krot_cm.__exit__(None, None, None)

            # ---- Phase C: q-proj (+bq) -> qTp_sb ----
            qTp_cm = tc.tile_pool(name="qTp", bufs=1)
            qTpp = qTp_cm.__enter__()
            qTp_sb = qTpp.tile([P, HO, SQ], BF16, name="qTp_sb")        # 32KB
            with tc.tile_pool(name="wq", bufs=2) as wqp:
                for qg in range(4):
                    wqq = wqp.tile([P, HO, 512], BF16, name="wq_q")
                    nc.sync.dma_start(out=wqq[:], in_=strip_ap(dram["wq"], qg * 512, 512))
                    for otl in range(4):
                        ot = qg * 4 + otl
                        for qc in range(2):
                            ps = psum.tile([P, 512], F32, name="ps_mm")
                            for h in range(HO):
                                nc.tensor.matmul(
                                    ps[:], wqq[:, h, otl * P:(otl + 1) * P],
                                    qrot_sb[:, h, qc * 512:(qc + 1) * 512],
                                    start=(h == 0), stop=(h == HO - 1))
                            nc.scalar.activation(
                                qTp_sb[:, ot, qc * 512:(qc + 1) * 512],
                                ps[:], Ident, bias=bias_sb["bq"][:, ot:ot + 1])
            qrot_cm.__exit__(None, None, None)

            # ---- Phase D: scores -> exp -> den (DMA-free) ----
            expT_cm = tc.tile_pool(name="expT", bufs=1, side="right")
            expTp = expT_cm.__enter__()
            expT = expTp.tile([P, SK // P, SQ], BF16, name="expT")      # 32KB
            with tc.tile_pool(name="p4den", bufs=1, space="PSUM") as p4den, \
                 tc.tile_pool(name="p4m", bufs=2) as p4m:
                _den = p4den.tile([1, 1024], F32, name="den")
                den_ps = [_den[:, 0:512], _den[:, 512:1024]]
                for kt in range(SK // P):
                    pss = [psum.tile([P, 512], F32, name="ps_mm") for _ in range(2)]
                    for o in range(HO):
                        for qc in range(2):
                            nc.tensor.matmul(
                                pss[qc][:], kTp_sb[:, o, kt * P:(kt + 1) * P],
                                qTp_sb[:, o, qc * 512:(qc + 1) * 512],
                                start=(o == 0), stop=(o == HO - 1))
                    for qc in range(2):
                        esl = expT[:, kt, qc * 512:(qc + 1) * 512]
                        nc.scalar.activation(esl, pss[qc][:], Exp, scale=SCALE)
                        nc.tensor.matmul(den_ps[qc][:], ones_col[:], esl,
                                         start=(kt == 0), stop=(kt == SK // P - 1))
                for qc in range(2):
                    rec = p4m.tile([1, 512], F32R, name="rec")
                    with nc.allow_low_precision("fp32r is 4-byte; feeds PE broadcast"):
                        nc.vector.reciprocal(rec[:], den_ps[qc][:])
                    bc = psum.tile([P, 512], F32, name="ps_mm")
                    nc.tensor.matmul(bc[:], ones_row[:], rec[:], start=True, stop=True)
                    nc.vector.tensor_copy(recip_bc[:, qc * 512:(qc + 1) * 512], bc[:])
            qTp_cm.__exit__(None, None, None)

            # ---- Phase E: context ----
            ctxT_cm = tc.tile_pool(name="ctxT", bufs=1)
            ctxTp = ctxT_cm.__enter__()
            ctxT = ctxTp.tile([P, HO, SQ], BF16, name="ctxT")           # 32KB
            with tc.tile_pool(name="p5s", bufs=4) as p5s:
                for ot in range(HO):
                    vstrip = ld_strip(p5s, v_st, ot * P, P, "v_strip")
                    pss = [psum.tile([P, 512], F32, name="ps_mm") for _ in range(2)]
                    for kt in range(SK // P):
                        for qc in range(2):
                            nc.tensor.matmul(
                                pss[qc][:], vstrip[:, kt, :],
                                expT[:, kt, qc * 512:(qc + 1) * 512],
                                start=(kt == 0), stop=(kt == SK // P - 1))
                    for qc in range(2):
                        csl = ctxT[:, ot, qc * 512:(qc + 1) * 512]
                        nc.vector.tensor_mul(csl, pss[qc][:],
                                             recip_bc[:, qc * 512:(qc + 1) * 512])
                        nc.vector.tensor_scalar_add(csl, csl,
                                                    bias_sb["bv"][:, ot:ot + 1])
            expT_cm.__exit__(None, None, None)
            kTp_cm.__exit__(None, None, None)

            # ---- Phase F: output projection ----
            with tc.tile_pool(name="p6s", bufs=3) as p6s, \
                 tc.tile_pool(name="p6o", bufs=4) as p6o:
                for mt in range(HO):
                    wstrip = ld_strip(p6s, dram["wo"], mt * P, P, "wo_strip")
                    pss = [psum.tile([P, 512], F32, name="ps_mm") for _ in range(2)]
                    for o in range(HO):
                        for qc in range(2):
                            nc.tensor.matmul(
                                pss[qc][:], wstrip[:, o, :],
                                ctxT[:, o, qc * 512:(qc + 1) * 512],
                                start=(o == 0), stop=(o == HO - 1))
                    for qc in range(2):
                        outt = p6o.tile([P, 512], F32, name="outt")
                        nc.scalar.activation(outt[:], pss[qc][:], Ident,
                                             bias=bias_sb["bo"][:, mt:mt + 1])
                        nc.sync.dma_start(
                            out=outT[mt * P:(mt + 1) * P, qc * 512:(qc + 1) * 512],
                            in_=outt[:])
            ctxT_cm.__exit__(None, None, None)
    return nc


def _build():
    global _PROG
    if _PROG is not None:
        return _PROG
    import concourse.bass as bass  # noqa: F401
    import concourse.mybir as mybir
    import concourse.tile as tile
    from concourse import bacc

    nc = bacc.Bacc("TRN2", target_bir_lowering=False, debug=False)
    _emit(nc, tile, mybir)
    nc.compile()
    _PROG = nc
    return nc


def _rope_tables():
    inv_freq = 1.0 / (10000.0 ** (np.arange(0, H, 2, dtype=np.float32) / H))
    t = np.arange(S, dtype=np.float32)
    freqs = np.outer(t, inv_freq).astype(np.float32)      # [S, H/2]
    cosT = np.ascontiguousarray(np.cos(freqs).T.astype(np.float16))  # [H/2, S]
    sinT = np.ascontiguousarray(np.sin(freqs).T.astype(np.float16))
    return cosT, sinT


def kernel(**inputs):
    nc = _build()
    from concourse.bass_utils import run_bass_kernel_spmd
    import ml_dtypes

    BF = ml_dtypes.bfloat16
    q = np.asarray(inputs["query"], dtype=np.float32)
    k = np.asarray(inputs["key"], dtype=np.float32)
    v = np.asarray(inputs["value"], dtype=np.float32)
    cosT, sinT = _rope_tables()
    wT = {n: np.ascontiguousarray(np.asarray(inputs[n], dtype=np.float32).T.astype(BF))
          for n in ("Wq", "Wk", "Wv", "Wo")}
    bias = {n: np.ascontiguousarray(np.asarray(inputs[n], dtype=np.float32))
            for n in ("bq", "bk", "bv", "bo")}
    ones_col = np.ones((P, 1), BF)
    ones_row = np.ones((1, P), np.float32)

    in_maps = []
    for c in range(NC_):
        b, half = divmod(c, 2)
        sl = slice(half * SQ, (half + 1) * SQ)
        in_maps.append({
            "qT": np.ascontiguousarray(q[b].T[:, sl].astype(BF)),
            "kT": np.ascontiguousarray(k[b].T.astype(BF)),
            "vT": np.ascontiguousarray(v[b].T.astype(BF)),
            "cos_q": np.ascontiguousarray(cosT[:, sl]),
            "sin_q": np.ascontiguousarray(sinT[:, sl]),
            "cos_k": cosT, "sin_k": sinT,
            "wq": wT["Wq"], "wk": wT["Wk"], "wv": wT["Wv"], "wo": wT["Wo"],
            "bq": bias["bq"], "bk": bias["bk"], "bv": bias["bv"], "bo": bias["bo"],
            "ones_col": ones_col, "ones_row": ones_row,
        })

    res = run_bass_kernel_spmd(nc, in_maps, core_ids=list(range(NC_)), trace=_TRACE)
    global LAST_RES
    LAST_RES = res
    out = np.empty((B, S, H), np.float32)
    for c in range(NC_):
        b, half = divmod(c, 2)
        out[b, half * SQ:(half + 1) * SQ, :] = res.results[c]["outT"].T
    return out



# revision 5
# speedup vs baseline: 1.0940x; 1.0940x over previous
"""Trainium2 Bass kernel for nn_CrossModalAttention (B=4, S=2048, H=2048, single head).

Sharding: 8 cores = 4 batches x 2 sequence-halves, fully balanced (no duplicated
projections). Core c handles batch b=c//2: query-half h=c%2 for q-proj/attention/
o-proj AND key-half h for k/v projections. The pair exchanges its kTp / v' shards
via pairwise AllGather (replica_groups [[0,1],[2,3],..]) which runs on the cc
cores (overlaps compute). AG output layout = [even_rank_shard, odd_rank_shard] =
keys in natural order; softmax is key-permutation invariant anyway.

Per-core PE work: 6 x 8.6 GFLOP = 51.5 GFLOP (vs 68.7 duplicated baseline).

Single-core dataflow, all bf16 staging:
  A  kTp' = WkT.T @ krot + bk  -> k_bounce (HBM) -> AG_k -> kTp_all [2H, 1024]
     [k-rope pipelined inline on DVE at phase start]
  B  v'   = value @ WvT        -> v_bounce (HBM) -> AG_v -> v_all [2048, H]
     [q-rope on DVE overlaps]
  C  qT'  = WqT.T @ qrot + bq  -> qTp_sb (SBUF)
  D  expT = exp(scale * kTp_all-strips @ qTp); den via PE ones-matmul
  E  ctxT = (v_all-strips @ expT) * (1/den) + bv
  F  outT = WoT.T @ ctxT + bo -> HBM

cos/sin tables: query-half h and key-half h cover the same positions -> one
table pair serves both ropes, kept in SBUF across A..B.
"""

import sys

for _p in ("/opt/trn_rl_repo",):
    if _p not in sys.path:
        sys.path.append(_p)

import numpy as np

B, S, H = 4, 2048, 2048
P = 128
HO = H // P            # 16 h-tiles
SQ = S // 2            # 1024 query cols per core
SKL = S // 2           # 1024 local key rows per core
SK = S                 # 2048 keys in attention (post-gather)
NC_ = 8
SCALE = 1.0 / float(np.sqrt(H))
RG_PAIRS = [[0, 1], [2, 3], [4, 5], [6, 7]]

_PROG = None
_TRACE = False
LAST_RES = None


def _emit(nc, tile, mybir):
    F32 = mybir.dt.float32
    F16 = mybir.dt.float16
    F32R = mybir.dt.float32r
    BF16 = mybir.dt.bfloat16
    Exp = mybir.ActivationFunctionType.Exp
    Ident = mybir.ActivationFunctionType.Identity
    Bypass = mybir.AluOpType.bypass

    dram = {}
    def din(name, shape, dt=F32):
        dram[name] = nc.dram_tensor(name, list(shape), dt, kind="ExternalInput").ap()
    din("qT", (H, SQ), BF16)
    din("kT", (H, SKL), BF16)
    din("vT", (H, SKL), BF16)
    din("cos_h", (H // 2, SQ), F16); din("sin_h", (H // 2, SQ), F16)
    din("wq", (H, H), BF16); din("wk", (H, H), BF16)
    din("wv", (H, H), BF16); din("wo", (H, H), BF16)
    din("bq", (H,)); din("bk", (H,)); din("bv", (H,)); din("bo", (H,))
    din("ones_col", (P, 1), BF16); din("ones_row", (1, P), F32R)
    outT = nc.dram_tensor("outT", [H, SQ], F32, kind="ExternalOutput").ap()

    def strip_ap(src2d, col0, width):
        return src2d[:, col0:col0 + width].rearrange("(o p) s -> p o s", p=P)

    def ld_strip(pool, src2d, col0, width, tag):
        rows = src2d.shape[0]
        t = pool.tile([P, rows // P, width], src2d.dtype, name=tag)
        nc.sync.dma_start(out=t[:], in_=strip_ap(src2d, col0, width))
        return t

    with tile.TileContext(nc) as tc:
        from contextlib import ExitStack
        with ExitStack() as ctx:
            glob = ctx.enter_context(tc.tile_pool(name="glob", bufs=1))
            dstage = ctx.enter_context(tc.tile_pool(name="dram", bufs=1, space="DRAM"))
            psum = ctx.enter_context(tc.tile_pool(name="psum", bufs=6, space="PSUM"))

            k_bounce = dstage.tile([H, SKL], BF16, name="k_bounce")
            kTp_all = dstage.tile([2 * H, SKL], BF16, name="kTp_all")
            v_bounce = dstage.tile([SKL, H], BF16, name="v_bounce")
            v_all = dstage.tile([SK, H], BF16, name="v_all")

            bias_sb = {}
            for bn in ("bq", "bk", "bv", "bo"):
                bias_sb[bn] = glob.tile([P, HO], F32, name=f"{bn}_sb")
            ones_col = glob.tile([P, 1], BF16, name="ones_col")
            ones_row = glob.tile([1, P], F32R, name="ones_row")
            recip_bc = glob.tile([P, SQ], F32, name="recip_bc")

            def load_globals():
                for bn in ("bq", "bk", "bv", "bo"):
                    nc.sync.dma_start(out=bias_sb[bn][:],
                                      in_=dram[bn].rearrange("(t p) -> p t", p=P))
                nc.sync.dma_start(out=ones_col[:], in_=dram["ones_col"])
                nc.sync.dma_start(out=ones_row[:], in_=dram["ones_row"])

            HH = HO // 2
            lo, hi = slice(0, HH), slice(HH, HO)

            def rope_chunk(dst, xsrc, cs_sb, sn_sb, pool, tag, c0, cw):
                # dst[:, :, c0:c0+cw] = rope(x) using SBUF-resident cos/sin
                x = pool.tile([P, HO, cw], BF16, name=f"{tag}_in")
                nc.sync.dma_start(out=x[:], in_=strip_ap(xsrc, c0, cw))
                cs = cs_sb[:, :, c0:c0 + cw]
                sn = sn_sb[:, :, c0:c0 + cw]
                tmp = pool.tile([P, HH, cw], BF16, name=f"{tag}_tmp")
                d = dst[:, :, c0:c0 + cw]
                nc.vector.tensor_mul(d[:, lo, :], x[:, lo, :], cs)
                nc.vector.tensor_mul(tmp[:], x[:, hi, :], sn)
                nc.vector.tensor_sub(d[:, lo, :], d[:, lo, :], tmp[:])
                nc.vector.tensor_mul(d[:, hi, :], x[:, hi, :], cs)
                nc.vector.tensor_mul(tmp[:], x[:, lo, :], sn)
                nc.vector.tensor_add(d[:, hi, :], d[:, hi, :], tmp[:])

            # cos/sin tables in SBUF for the whole A..B window (k- and q-rope
            # use the same sequence positions on this core)
            cssn_cm = tc.tile_pool(name="cssn", bufs=1, side="right")
            cssnp = cssn_cm.__enter__()
            cs_sb = cssnp.tile([P, HH, SQ], F16, name="cs_sb")
            sn_sb = cssnp.tile([P, HH, SQ], F16, name="sn_sb")
            for c0 in range(0, SQ, 256):  # chunked so rope chunk 0 starts early
                nc.sync.dma_start(out=cs_sb[:, :, c0:c0 + 256],
                                  in_=strip_ap(dram["cos_h"], c0, 256))
                nc.sync.dma_start(out=sn_sb[:, :, c0:c0 + 256],
                                  in_=strip_ap(dram["sin_h"], c0, 256))

            # vT_sb prefetched during phase A so v-proj starts without a bubble
            vT_cm = tc.tile_pool(name="vT", bufs=1)
            vTp = vT_cm.__enter__()
            vT_sb = vTp.tile([P, HO, SKL], BF16, name="vT_sb")
            for c0 in range(0, SKL, 512):
                nc.sync.dma_start(out=vT_sb[:, :, c0:c0 + 512],
                                  in_=strip_ap(dram["vT"], c0, 512))

            krot_cm = tc.tile_pool(name="krot", bufs=1)
            krotp = krot_cm.__enter__()
            krot_sb = krotp.tile([P, HO, SKL], BF16, name="krot_sb")   # 32KB

            # ---- Phase A: k-rope (inline) + k-proj (+bk) -> k_bounce ----
            with tc.tile_pool(name="wk", bufs=2) as wkp, \
                 tc.tile_pool(name="kev", bufs=4) as kev, \
                 tc.tile_pool(name="krope", bufs=3) as krope:
                load_globals()
                for c0 in range(0, SKL, P):
                    rope_chunk(krot_sb, dram["kT"], cs_sb, sn_sb, krope, "kr", c0, P)
                for ob in range(HO):
                    wks = ld_strip(wkp, dram["wk"], ob * P, P, "wk_s")
                    for kc in range(SKL // 512):        # 2
                        ps = psum.tile([P, 512], F32, name="ps_mm")
                        for h in range(HO):
                            nc.tensor.matmul(
                                ps[:], wks[:, h, :],
                                krot_sb[:, h, kc * 512:(kc + 1) * 512],
                                start=(h == 0), stop=(h == HO - 1))
                        ev = kev.tile([P, 512], BF16, name="k_ev")
                        nc.scalar.activation(ev[:], ps[:], Ident,
                                             bias=bias_sb["bk"][:, ob:ob + 1])
                        nc.sync.dma_start(
                            out=k_bounce[ob * P:(ob + 1) * P, kc * 512:(kc + 1) * 512],
                            in_=ev[:])
            krot_cm.__exit__(None, None, None)

            nc.gpsimd.collective_compute(
                "AllGather", Bypass, replica_groups=RG_PAIRS,
                ins=[k_bounce.opt()], outs=[kTp_all.opt()])

            # qrot: written in B (DVE), read in C
            qrot_cm = tc.tile_pool(name="qrot", bufs=1, side="right")
            qrotp = qrot_cm.__enter__()
            qrot_sb = qrotp.tile([P, HO, SQ], BF16, name="qrot_sb")    # 32KB

            # ---- Phase B: v-proj -> v_bounce + q-rope on DVE ----
            with tc.tile_pool(name="wv", bufs=2) as wvp, \
                 tc.tile_pool(name="vev", bufs=4) as vev, \
                 tc.tile_pool(name="qrope", bufs=3) as qrope:
                for c0 in range(0, SQ, P):
                    rope_chunk(qrot_sb, dram["qT"], cs_sb, sn_sb, qrope, "qr", c0, P)
                for oc in range(4):                     # 512-col groups of Wv
                    wvq = wvp.tile([P, HO, 512], BF16, name="wv_q")
                    nc.sync.dma_start(out=wvq[:], in_=strip_ap(dram["wv"], oc * 512, 512))
                    for st in range(SKL // P):          # 8 strips
                        ps = psum.tile([P, 512], F32, name="ps_mm")
                        for h in range(HO):
                            nc.tensor.matmul(
                                ps[:], vT_sb[:, h, st * P:(st + 1) * P],
                                wvq[:, h, :],
                                start=(h == 0), stop=(h == HO - 1))
                        ev = vev.tile([P, 512], BF16, name="v_ev")
                        nc.scalar.copy(ev[:], ps[:])
                        nc.sync.dma_start(
                            out=v_bounce[st * P:(st + 1) * P, oc * 512:(oc + 1) * 512],
                            in_=ev[:])
            vT_cm.__exit__(None, None, None)

            nc.gpsimd.collective_compute(
                "AllGather", Bypass, replica_groups=RG_PAIRS,
                ins=[v_bounce.opt()], outs=[v_all.opt()])

            # ---- Phase C: q-proj (+bq) -> qTp_sb ----
            qTp_cm = tc.tile_pool(name="qTp", bufs=1)
            qTpp = qTp_cm.__enter__()
            qTp_sb = qTpp.tile([P, HO, SQ], BF16, name="qTp_sb")       # 32KB
            with tc.tile_pool(name="wq", bufs=2) as wqp:
                for qg in range(8):                     # 256-col groups of Wq
                    wqq = wqp.tile([P, HO, 256], BF16, name="wq_q")
                    nc.sync.dma_start(out=wqq[:], in_=strip_ap(dram["wq"], qg * 256, 256))
                    for otl in range(2):
                        ot = qg * 2 + otl
                        for qc in range(2):
                            ps = psum.tile([P, 512], F32, name="ps_mm")
                            for h in range(HO):
                                nc.tensor.matmul(
                                    ps[:], wqq[:, h, otl * P:(otl + 1) * P],
                                    qrot_sb[:, h, qc * 512:(qc + 1) * 512],
                                    start=(h == 0), stop=(h == HO - 1))
                            nc.scalar.activation(
                                qTp_sb[:, ot, qc * 512:(qc + 1) * 512],
                                ps[:], Ident, bias=bias_sb["bq"][:, ot:ot + 1])
            qrot_cm.__exit__(None, None, None)
            cssn_cm.__exit__(None, None, None)

            # ---- Phase D: scores -> exp -> den (kTp strips from AG output) ----
            expT_cm = tc.tile_pool(name="expT", bufs=1, side="right")
            expTp = expT_cm.__enter__()
            expT = expTp.tile([P, SK // P, SQ], BF16, name="expT")     # 32KB
            with tc.tile_pool(name="p4den", bufs=1, space="PSUM") as p4den, \
                 tc.tile_pool(name="p4m", bufs=2) as p4m, \
                 tc.tile_pool(name="kst", bufs=3) as kst:
                _den = p4den.tile([1, 1024], F32, name="den")
                den_ps = [_den[:, 0:512], _den[:, 512:1024]]
                for kt in range(SK // P):               # 16 key strips
                    half, c0 = divmod(kt * P, SKL)
                    ksrc = kTp_all[half * H:(half + 1) * H, :]
                    kstrip = ld_strip(kst, ksrc, c0, P, "k_strip")
                    pss = [psum.tile([P, 512], F32, name="ps_mm") for _ in range(2)]
                    for o in range(HO):
                        for qc in range(2):
                            nc.tensor.matmul(
                                pss[qc][:], kstrip[:, o, :],
                                qTp_sb[:, o, qc * 512:(qc + 1) * 512],
                                start=(o == 0), stop=(o == HO - 1))
                    for qc in range(2):
                        esl = expT[:, kt, qc * 512:(qc + 1) * 512]
                        nc.scalar.activation(esl, pss[qc][:], Exp, scale=SCALE)
                        nc.tensor.matmul(den_ps[qc][:], ones_col[:], esl,
                                         start=(kt == 0), stop=(kt == SK // P - 1))
                for qc in range(2):
                    rec = p4m.tile([1, 512], F32R, name="rec")
                    with nc.allow_low_precision("fp32r is 4-byte; feeds PE broadcast"):
                        nc.vector.reciprocal(rec[:], den_ps[qc][:])
                    bc = psum.tile([P, 512], F32, name="ps_mm")
                    nc.tensor.matmul(bc[:], ones_row[:], rec[:], start=True, stop=True)
                    nc.vector.tensor_copy(recip_bc[:, qc * 512:(qc + 1) * 512], bc[:])
            qTp_cm.__exit__(None, None, None)

            # ---- Phase E: context (v strips from AG output) ----
            ctxT_cm = tc.tile_pool(name="ctxT", bufs=1)
            ctxTp = ctxT_cm.__enter__()
            ctxT = ctxTp.tile([P, HO, SQ], BF16, name="ctxT")          # 32KB
            with tc.tile_pool(name="p5s", bufs=3) as p5s:
                for ot in range(HO):
                    vstrip = ld_strip(p5s, v_all, ot * P, P, "v_strip")
                    pss = [psum.tile([P, 512], F32, name="ps_mm") for _ in range(2)]
                    for kt in range(SK // P):
                        for qc in range(2):
                            nc.tensor.matmul(
                                pss[qc][:], vstrip[:, kt, :],
                                expT[:, kt, qc * 512:(qc + 1) * 512],
                                start=(kt == 0), stop=(kt == SK // P - 1))
                    for qc in range(2):
                        csl = ctxT[:, ot, qc * 512:(qc + 1) * 512]
                        nc.vector.tensor_mul(csl, pss[qc][:],
                                             recip_bc[:, qc * 512:(qc + 1) * 512])
                        nc.vector.tensor_scalar_add(csl, csl,
                                                    bias_sb["bv"][:, ot:ot + 1])
            expT_cm.__exit__(None, None, None)

            # ---- Phase F: output projection ----
            with tc.tile_pool(name="p6s", bufs=3) as p6s, \
                 tc.tile_pool(name="p6o", bufs=4) as p6o:
                for mt in range(HO):
                    wstrip = ld_strip(p6s, dram["wo"], mt * P, P, "wo_strip")
                    pss = [psum.tile([P, 512], F32, name="ps_mm") for _ in range(2)]
                    for o in range(HO):
                        for qc in range(2):
                            nc.tensor.matmul(
                                pss[qc][:], wstrip[:, o, :],
                                ctxT[:, o, qc * 512:(qc + 1) * 512],
                                start=(o == 0), stop=(o == HO - 1))
                    for qc in range(2):
                        outt = p6o.tile([P, 512], F32, name="outt")
                        nc.scalar.activation(outt[:], pss[qc][:], Ident,
                                             bias=bias_sb["bo"][:, mt:mt + 1])
                        nc.sync.dma_start(
                            out=outT[mt * P:(mt + 1) * P, qc * 512:(qc + 1) * 512],
                            in_=outt[:])
            ctxT_cm.__exit__(None, None, None)
    return nc


def _build():
    global _PROG
    if _PROG is not None:
        return _PROG
    import concourse.bass as bass  # noqa: F401
    import concourse.mybir as mybir
    import concourse.tile as tile
    from concourse import bacc

    nc = bacc.Bacc("TRN2", target_bir_lowering=False, debug=False, num_devices=NC_)
    _emit(nc, tile, mybir)
    nc.compile()
    _PROG = nc
    return nc


def _rope_tables():
    inv_freq = 1.0 / (10000.0 ** (np.arange(0, H, 2, dtype=np.float32) / H))
    t = np.arange(S, dtype=np.float32)
    freqs = np.outer(t, inv_freq).astype(np.float32)      # [S, H/2]
    cosT = np.ascontiguousarray(np.cos(freqs).T.astype(np.float16))  # [H/2, S]
    sinT = np.ascontiguousarray(np.sin(freqs).T.astype(np.float16))
    return cosT, sinT


def kernel(**inputs):
    nc = _build()
    from concourse.bass_utils import run_bass_kernel_spmd
    import ml_dtypes

    BF = ml_dtypes.bfloat16
    q = np.asarray(inputs["query"], dtype=np.float32)
    k = np.asarray(inputs["key"], dtype=np.float32)
    v = np.asarray(inputs["value"], dtype=np.float32)
    cosT, sinT = _rope_tables()
    wT = {n: np.ascontiguousarray(np.asarray(inputs[n], dtype=np.float32).T.astype(BF))
          for n in ("Wq", "Wk", "Wv", "Wo")}
    bias = {n: np.ascontiguousarray(np.asarray(inputs[n], dtype=np.float32))
            for n in ("bq", "bk", "bv", "bo")}
    ones_col = np.ones((P, 1), BF)
    ones_row = np.ones((1, P), np.float32)

    in_maps = []
    for c in range(NC_):
        b, half = divmod(c, 2)
        sl = slice(half * SQ, (half + 1) * SQ)
        in_maps.append({
            "qT": np.ascontiguousarray(q[b].T[:, sl].astype(BF)),
            "kT": np.ascontiguousarray(k[b].T[:, sl].astype(BF)),
            "vT": np.ascontiguousarray(v[b].T[:, sl].astype(BF)),
            "cos_h": np.ascontiguousarray(cosT[:, sl]),
            "sin_h": np.ascontiguousarray(sinT[:, sl]),
            "wq": wT["Wq"], "wk": wT["Wk"], "wv": wT["Wv"], "wo": wT["Wo"],
            "bq": bias["bq"], "bk": bias["bk"], "bv": bias["bv"], "bo": bias["bo"],
            "ones_col": ones_col, "ones_row": ones_row,
        })

    res = run_bass_kernel_spmd(nc, in_maps, core_ids=list(range(NC_)), trace=_TRACE)
    global LAST_RES
    LAST_RES = res
    out = np.empty((B, S, H), np.float32)
    for c in range(NC_):
        b, half = divmod(c, 2)
        out[b, half * SQ:(half + 1) * SQ, :] = res.results[c]["outT"].T
    return out
